# revision 31
# baseline (speedup 1.0000x reference)
"""Trainium2 Bass kernel for a dense attention layer.

Problem (hardcoded): N=4, S=T=4096, D=256, fp32.
  q = query @ Wq.T + bq ; k = key @ Wk.T + bk ; v = value @ Wv.T + bv
  y = softmax(q @ k.T / sqrt(D)) @ v
Sharding: 8 cores = (batch n in 0..3) x (S-half h in 0..1). Each core gets
its Q shard [2048, 256] plus the full K/V [4096, 256] of its batch; pure
SPMD, no collectives.

Math structure (all folds host-side):
  - BOTH projections on the q side collapse into one: scores^T = K_raw qM
    with qM = M^T x + b', M = (Wq^T/sqrt(D)) Wk, b' = (bq/sqrt(D)) Wk. The
    bk-dependent score term is constant along the softmax axis and drops.
  - softmax is unnormalized exp (scores ~ N(0,1), no max subtraction); the
    row-sum rides as a ones-column through the V projection and the divide
    happens after the PV matmul.
  - scores are computed transposed ([t, s] tiles) so PV needs no
    transposes; V's projection doubles as its transpose into [t, d'].
All matmuls are fp16 x fp16 -> fp32 PSUM at full PE rate (measured issue
spacing 110ns/258-col, 216ns/512-col). Output is stored fp16 (rel-err
budget 2e-2; fp16 adds ~2e-4) and upcast on host.

Schedule (times ~us, after the fixed ~7.2us framework preamble):
  - DMAs are split so the first scores' gate is only ~0.5MB (M tiles, k/q
    first chunks, b'), spread one-per-queue so they land in parallel ~10;
    everything else is ordered by first use across the sync/scalar HWDGE
    queues and the gpsimd SWDGE queue (measured ~320GB/s aggregate, all
    input resident by ~27).
  - 7 dep-free fp16 warmup matmuls bridge the PE from preamble-end to the
    first projection so the HAM clock-gate releases once and stays
    released; a dummy exp pulls the 1.3us ACT_TABLE_LOAD into the head.
  - Phase A: qM(c0) + all 16 score pairs of chunk 0, with the 32 V
    projections riding on the later pairs (paced to vin arrival, and so
    the Scalar engine's 1.1us exps never back up the 2-deep score PSUM
    pool).
  - Main loop: scores(c+1) stagger against y(c) — the one-chunk stagger
    keeps exp latency off the PE critical path (pairing same-chunk
    scores+y measured +16us). qM(c2/c3) ride via the score PSUM pool
    since the y pool's 4 bufs hold the accumulators.
  - c3: PV st-major so each output tile's finalize+DMA overlaps the
    remaining matmuls (the out-DMA receipt alone is ~2.5us).
"""

import numpy as np

import concourse.bacc as bacc
import concourse.mybir as mybir
import concourse.tile as tile
from concourse.bass_utils import run_bass_kernel_spmd

# ---- problem constants (per core) ----
D = 256           # embed dim
S = 2048          # local query rows (S_global=4096 split in 2)
T = 4096          # key/value rows (full batch)
SC = 512          # s-chunk width for the scores/exp stage
N_SC = S // SC    # 4 s-chunks
N_TT = T // 128   # 32 t-tiles
N_TP = N_TT // 2  # 16 t-tile pairs (2 score tiles share one psum/exp tile)
DV = D + 2        # v free dim incl. ones column (+1 pad for even free dim)

F32 = mybir.dt.float32
F16 = mybir.dt.float16
EXP = mybir.ActivationFunctionType.Exp
IDT = mybir.ActivationFunctionType.Identity

_CACHE = {}


def _build():
    nc = bacc.Bacc("TRN2", target_bir_lowering=False, debug=False)

    qT = nc.dram_tensor("qT", [D, S], F16, kind="ExternalInput")    # (d, s)
    kT = nc.dram_tensor("kT", [D, T], F16, kind="ExternalInput")    # (d, t)
    vT = nc.dram_tensor("vT", [D, T], F16, kind="ExternalInput")    # (d, t)
    # M tiles for the q-side fold: cols [(dk,d)] = M[d-half, dk-half]
    wM = nc.dram_tensor("wM", [128, 512], F16, kind="ExternalInput")
    wv = nc.dram_tensor("wv", [128, 2 * DV], F16, kind="ExternalInput")
    bq2 = nc.dram_tensor("bq2", [128, 2], F32, kind="ExternalInput")
    bv = nc.dram_tensor("bv", [128, DV], F32, kind="ExternalInput")
    out = nc.dram_tensor("out", [S, D], F16, kind="ExternalOutput")

    with tile.TileContext(nc) as tc:
        _emit(nc, tc, qT, kT, vT, wM, wv, bq2, bv, out)
    nc.compile()
    return nc


def _emit(nc, tc, qT, kT, vT, wM, wv, bq2, bv, out):
    from contextlib import ExitStack

    with ExitStack() as ctx:
        consts = ctx.enter_context(tc.tile_pool(name="consts", bufs=1))
        persist = ctx.enter_context(tc.tile_pool(name="persist", bufs=1))
        pool_in = ctx.enter_context(tc.tile_pool(name="inputs", bufs=1))
        pool_exp = ctx.enter_context(tc.tile_pool(name="exp", bufs=18))
        pool_y = ctx.enter_context(tc.tile_pool(name="ysb", bufs=4))
        ps_sc = ctx.enter_context(tc.tile_pool(name="ps_sc", bufs=2, space="PSUM"))
        ps_y = ctx.enter_context(tc.tile_pool(name="ps_y", bufs=4, space="PSUM"))

        wM_t = consts.tile([128, 512], F16, tag="wM", name="wM")
        wv_t = consts.tile([128, 2 * DV], F16, tag="wv", name="wv")
        bq_t = consts.tile([128, 2], F32, tag="bq", name="bq")
        bv_t = consts.tile([128, DV], F32, tag="bv", name="bv")
        kin = [pool_in.tile([128, T], F16, tag=f"kin{d}", name=f"kin{d}")
               for d in range(2)]
        qin = [pool_in.tile([128, S], F16, tag=f"qin{d}", name=f"qin{d}")
               for d in range(2)]
        vin = [pool_in.tile([128, T], F16, tag=f"vin{d}", name=f"vin{d}")
               for d in range(2)]

        # DMA choreography, ordered by first use within each queue; the
        # critical head items (wM / kin-c0a+qA0 / bq+qA1) ride first, one
        # per queue, so they land in parallel. Measured aggregate stream
        # rate with these descriptor sizes is ~320GB/s; everything lands
        # by ~27us while phase A computes until ~30us.
        # Everything needed in the first ~27us goes on the two HWDGE
        # queues (~160GB/s each); the gpsimd SWDGE lane is slow under
        # HWDGE contention (~25GB/s measured) and only carries the tiny
        # biases, wv, and the late q tail. q chunk A covers cols 0:1024
        # so both qM(c0) and qM(c1) ride on it. scalar's issues 5/6
        # credit-block the Scalar engine briefly ~10-12us, before exps.
        # The gpsimd SWDGE queue delivers only ~15GB/s while the HWDGE
        # queues are busy (measured: an item behind 130KB landed at 26us)
        # so ONLY data needed after ~35us may ride it. All weights/biases
        # and the first three kin chunks go HWDGE.
        # With the half-chunk stagger, phase A only consumes kin chunks
        # c0/c1 and ALL of vin (the V projections); kin c2/c3 aren't
        # needed until the first main-loop iteration (~30us), so vin goes
        # ahead of them in the queues.
        # The head (weights, q chunk A, kin c0/c1) is split into 128KB
        # pieces so completions arrive every ~1us: coarse 256KB chunks
        # left 2-3us arrival holes that re-throttled the HAM clock-gate.
        # scalar stays <=6 issues (credit waits end before the first exp);
        # sync (idle engine) absorbs the long FIFO. kin c2/c3 and qB are
        # only needed by the main loop (~30us+).
        KC = 1024
        nc.scalar.dma_start(wM_t[:], wM[:, :])
        nc.sync.dma_start(qin[0][:, 0:512], qT[0:128, 0:512])
        nc.gpsimd.dma_start(bq_t[:], bq2[:, :])
        nc.scalar.dma_start(qin[1][:, 0:512], qT[128:256, 0:512])
        nc.sync.dma_start(kin[0][:, 0:512], kT[0:128, 0:512])
        nc.scalar.dma_start(kin[1][:, 0:512], kT[128:256, 0:512])
        nc.sync.dma_start(kin[0][:, 512:KC], kT[0:128, 512:KC])
        nc.scalar.dma_start(kin[1][:, 512:KC], kT[128:256, 512:KC])
        nc.sync.dma_start(qin[0][:, 512:KC], qT[0:128, 512:KC])
        nc.scalar.dma_start(qin[1][:, 512:KC], qT[128:256, 512:KC])
        nc.sync.dma_start(kin[0][:, KC:KC + 512], kT[0:128, KC:KC + 512])
        nc.scalar.dma_start(kin[1][:, KC:KC + 512], kT[128:256, KC:KC + 512])
        nc.sync.dma_start(kin[0][:, KC + 512:2 * KC], kT[0:128, KC + 512:2 * KC])
        nc.sync.dma_start(kin[1][:, KC + 512:2 * KC], kT[128:256, KC + 512:2 * KC])
        nc.sync.dma_start(wv_t[:], wv[:, :])
        nc.sync.dma_start(bv_t[:], bv[:, :])
        nc.sync.dma_start(vin[1][:, 0:2048], vT[128:256, 0:2048])
        nc.sync.dma_start(vin[0][:, 0:2048], vT[0:128, 0:2048])
        nc.sync.dma_start(vin[1][:, 2048:T], vT[128:256, 2048:T])
        nc.sync.dma_start(vin[0][:, 2048:T], vT[0:128, 2048:T])
        nc.sync.dma_start(kin[0][:, 2 * KC:3 * KC], kT[0:128, 2 * KC:3 * KC])
        nc.sync.dma_start(kin[1][:, 2 * KC:3 * KC], kT[128:256, 2 * KC:3 * KC])
        nc.gpsimd.dma_start(kin[1][:, 3 * KC:T], kT[128:256, 3 * KC:T])
        nc.gpsimd.dma_start(kin[0][:, 3 * KC:T], kT[0:128, 3 * KC:T])
        nc.gpsimd.dma_start(qin[1][:, KC:S], qT[128:256, KC:S])
        nc.gpsimd.dma_start(qin[0][:, KC:S], qT[0:128, KC:S])

        # ---- PE warmup + ACT table preload ----
        warm = consts.tile([128, 512], F16, tag="warm", name="warm")
        nc.vector.memset(warm[:], 0.0)
        for _ in range(7):
            wps = ps_sc.tile([128, 512], F32, tag="ps", name="ps")
            nc.tensor.matmul(wps[:], warm[:, 0:128], warm[:], start=True,
                             stop=True)
        wexp = consts.tile([128, 8], F16, tag="wexp", name="wexp")
        nc.scalar.activation(wexp[:], warm[:, 0:8], EXP)

        def m_t(d, dk):
            c0 = (dk * 2 + d) * 128
            return wM_t[:, c0:c0 + 128]

        wv_s = [wv_t[:, 0:DV], wv_t[:, DV:2 * DV]]

        qM = [persist.tile([128, S], F16, tag=f"qM{d}", name=f"qM{d}")
              for d in range(2)]
        vs = persist.tile([128, N_TT * DV], F16, tag="vs", name="vs")

        # qM[dk, s] = sum_d M[d, dk] qT[d, s] + b'[dk]; epilogues on the
        # Vector engine so the Scalar engine stays exp-only. `pool` picks
        # the PSUM pool: ps_y before the y accumulators exist, ps_sc after
        # (ps_y's 4 bufs are all held by yps then).
        def qMproj(sc_i, pool):
            sl = slice(sc_i * SC, (sc_i + 1) * SC)
            for dk in range(2):
                ps = pool.tile([128, 512], F32,
                               tag="psv" if pool is ps_y else "ps", name="qMp")
                for d in range(2):
                    nc.tensor.matmul(ps[:], m_t(d, dk), qin[d][:, sl],
                                     start=(d == 0), stop=(d == 1))
                nc.vector.tensor_scalar_add(qM[dk][:, sl], ps[:],
                                            bq_t[:, dk:dk + 1])

        exp_tiles = {}

        def emit_scores_pair(c, tp):
            """Scores for t-tiles (2tp, 2tp+1) x s-chunk c -> one exp tile."""
            ssl = slice(c * SC, (c + 1) * SC)
            ps = ps_sc.tile([128, 2 * SC], F32, tag="ps", name="ps")
            for dk in (0, 1):
                for j in (0, 1):
                    tt = 2 * tp + j
                    half = slice(j * SC, (j + 1) * SC)
                    nc.tensor.matmul(
                        ps[:, half], kin[dk][:, tt * 128:(tt + 1) * 128],
                        qM[dk][:, ssl], start=(dk == 0), stop=(dk == 1))
            et = pool_exp.tile([128, 2 * SC], F16, tag="exp", name="exp")
            nc.scalar.activation(et[:], ps[:], EXP)
            exp_tiles[(c, tp)] = et

        def emit_vproj(tt):
            tsl = slice(tt * 128, (tt + 1) * 128)
            ps = ps_y.tile([128, DV], F32, tag="psv", name="psv")
            for d in range(2):
                nc.tensor.matmul(ps[:], vin[d][:, tsl], wv_s[d][:],
                                 start=(d == 0), stop=(d == 1))
            nc.vector.tensor_add(vs[:, tt * DV:(tt + 1) * DV], ps[:], bv_t[:])

        def emit_y_step(c, tp, yps):
            et = exp_tiles.pop((c, tp))
            for j in (0, 1):
                tt = 2 * tp + j
                for st in range(4):
                    nc.tensor.matmul(
                        yps[st][:],
                        et[:, j * SC + st * 128: j * SC + (st + 1) * 128],
                        vs[:, tt * DV:(tt + 1) * DV],
                        start=(tt == 0), stop=(tt == N_TT - 1))

        def finalize_y(c, yps):
            for st in range(4):
                s0 = c * SC + st * 128
                recip = pool_y.tile([128, 1], F32, tag="recip", name="recip")
                nc.vector.reciprocal(recip[:], yps[st][:, D:D + 1])
                y_sb = pool_y.tile([128, D], F16, tag="ysb", name="ysb")
                nc.vector.tensor_scalar_mul(y_sb[:], yps[st][:, 0:D],
                                            recip[:, 0:1])
                nc.sync.dma_start(out[s0:s0 + 128, :], y_sb[:])

        def _finalize_one(c, st, yp):
            s0 = c * SC + st * 128
            recip = pool_y.tile([128, 1], F32, tag="recip", name="recip")
            nc.vector.reciprocal(recip[:], yp[:, D:D + 1])
            y_sb = pool_y.tile([128, D], F16, tag="ysb", name="ysb")
            if st % 2 == 0:
                nc.vector.tensor_scalar_mul(y_sb[:], yp[:, 0:D], recip[:, 0:1])
            else:
                nc.scalar.activation(y_sb[:], yp[:, 0:D], IDT,
                                     scale=recip[:, 0:1])
            if st < 3:
                eng = [nc.sync, nc.scalar, nc.gpsimd][st]
                eng.dma_start(out[s0:s0 + 128, :], y_sb[:])
            else:
                nc.sync.dma_start(out[s0:s0 + 64, :], y_sb[0:64, :])
                nc.scalar.dma_start(out[s0 + 64:s0 + 128, :], y_sb[64:128, :])

        # ---- phase A: only the FIRST 8 pairs of chunk 0 ever run in the
        # latency-bound regime (scores<->exp round-trip through the 2-deep
        # PSUM pool is ~1.8us/pair; in the main loop the y-steps lengthen
        # the PE leg so the loop is PE-bound instead). The 32 V
        # projections ride here, paced to vin arrival. Dep-free ldweights
        # fillers keep the PE array active while A waits on DMA/exp so
        # the HAM clock-gate never re-throttles (a cold window halves the
        # PE clock for >=3.4us).
        def filler(n):
            for _ in range(n):
                nc.tensor.ldweights(warm[:, 0:128])

        qMproj(0, ps_y)
        filler(24)
        for tp in range(8):
            emit_scores_pair(0, tp)
            filler(4)
            if tp == 1:
                qMproj(1, ps_y)
                filler(6)
        for tt in range(N_TT):
            emit_vproj(tt)
            if tt % 2 == 1:
                filler(2)

        # ---- main loop, half-chunk stagger: iteration c emits chunk c's
        # tail pairs then chunk c+1's head pairs while consuming y(c) —
        # the 8-pair lead keeps exp latency off the PE path and no phase
        # anywhere runs scores alone. qM(c2/c3) ride via the score PSUM
        # pool since the y pool's 4 bufs hold the accumulators. ----
        for c in range(N_SC):
            yps = [ps_y.tile([128, DV], F32, tag="psv", name="psv")
                   for _ in range(4)]
            last = c == N_SC - 1
            for tp in range(N_TP - (2 if last else 0)):
                if tp < 8:
                    emit_scores_pair(c, tp + 8)
                elif c < N_SC - 1:
                    emit_scores_pair(c + 1, tp - 8)
                emit_y_step(c, tp, yps)
                if c == 0 and tp in (10, 12):
                    qMproj(2 + (tp - 10) // 2, ps_sc)
            if not last:
                finalize_y(c, yps)
            else:
                # last two tile-pairs st-major so each output tile's
                # finalize + DMA overlaps the remaining PV matmuls
                ets = [exp_tiles.pop((c, tp)) for tp in (14, 15)]
                for st in range(4):
                    for p, et in enumerate(ets):
                        for j in (0, 1):
                            tt = 28 + 2 * p + j
                            nc.tensor.matmul(
                                yps[st][:],
                                et[:, j * SC + st * 128: j * SC + (st + 1) * 128],
                                vs[:, tt * DV:(tt + 1) * DV],
                                start=False, stop=(tt == N_TT - 1))
                    _finalize_one(c, st, yps[st])


def _get_nc():
    if "nc" not in _CACHE:
        _CACHE["nc"] = _build()
    return _CACHE["nc"]


def _make_in_maps(inputs):
    query = np.asarray(inputs["query"], dtype=np.float32)
    key = np.asarray(inputs["key"], dtype=np.float32)
    value = np.asarray(inputs["value"], dtype=np.float32)
    Wq = np.asarray(inputs["Wq"], np.float64)
    bq = np.asarray(inputs["bq"], np.float64)
    Wk = np.asarray(inputs["Wk"], np.float64)
    Wv = np.asarray(inputs["Wv"], np.float32)
    bv = np.asarray(inputs["bv"], np.float32)
    scale = 1.0 / 16.0  # 1/sqrt(D)

    # q-side fold: scores^T = K_raw^T (M^T x + b')
    M = (Wq.T * scale) @ Wk            # [d, dk]
    bprime = (bq * scale) @ Wk         # [dk]

    wM_h = np.zeros((128, 512), np.float16)
    for dk in range(2):
        for d in range(2):
            c0 = (dk * 2 + d) * 128
            wM_h[:, c0:c0 + 128] = M[d * 128:(d + 1) * 128,
                                     dk * 128:(dk + 1) * 128]
    wv_h = np.zeros((128, 2 * DV), np.float16)
    wvT = Wv.T.astype(np.float16)      # [d, e]
    wv_h[:, 0:D] = wvT[0:128]
    wv_h[:, DV:DV + D] = wvT[128:256]
    bq_h = np.zeros((128, 2), np.float32)
    bq_h[:, 0] = bprime[0:128]
    bq_h[:, 1] = bprime[128:256]
    bv_h = np.zeros((128, DV), np.float32)
    bv_h[:, :D] = bv[None, :]
    bv_h[:, D] = 1.0

    in_maps = []
    for c in range(8):
        n, h = divmod(c, 2)
        in_maps.append({
            "qT": np.ascontiguousarray(
                query[n, h * S:(h + 1) * S, :].T).astype(np.float16),
            "kT": np.ascontiguousarray(key[n].T).astype(np.float16),
            "vT": np.ascontiguousarray(value[n].T).astype(np.float16),
            "wM": wM_h, "wv": wv_h, "bq2": bq_h, "bv": bv_h,
        })
    return in_maps


def kernel(query, key, value, Wq, bq, Wk, bk, Wv, bv):
    in_maps = _make_in_maps(dict(query=query, key=key, value=value, Wq=Wq,
                                 bq=bq, Wk=Wk, bk=bk, Wv=Wv, bv=bv))
    nc = _get_nc()
    res = run_bass_kernel_spmd(nc, in_maps, core_ids=list(range(8)))

    y = np.empty((4, 2 * S, D), np.float32)
    for c in range(8):
        n, h = divmod(c, 2)
        y[n, h * S:(h + 1) * S, :] = res.results[c]["out"].astype(np.float32)
    return y


if __name__ == "__main__":
    rng = np.random.default_rng(0)
    inputs = {
        "query": rng.standard_normal((4, 4096, 256), dtype=np.float32),
        "key": rng.standard_normal((4, 4096, 256), dtype=np.float32),
        "value": rng.standard_normal((4, 4096, 256), dtype=np.float32),
        "Wq": (rng.standard_normal((256, 256), dtype=np.float32) / 16),
        "bq": (rng.standard_normal(256, dtype=np.float32) / 16),
        "Wk": (rng.standard_normal((256, 256), dtype=np.float32) / 16),
        "bk": (rng.standard_normal(256, dtype=np.float32) / 16),
        "Wv": (rng.standard_normal((256, 256), dtype=np.float32) / 16),
        "bv": (rng.standard_normal(256, dtype=np.float32) / 16),
    }
    y = kernel(**inputs)
    print("ran ok", y.shape, y.dtype)
